# revision 18
# baseline (speedup 1.0000x reference)
"""GCN 2-layer (gcn_norm) SPMD Bass kernel for 8 TRN2 NeuronCores.

Strategy (node partition + edge partition by destination):
  - nodes sharded 6250/core; edges assigned to the core owning their dst.
  - layer math: out = dis * (sum_{e->v} dis[src]*h[src]) + dis^2*h_v + b
    with dis = deg^-1/2 (deg includes self-loop), h = x@W.
  - per layer: local projection -> scale by dis -> two half AllGathers
    (first/second half of each core's rows) into [25000,ch] tables in each
    core's HBM -> dma_gather rows for the core's edges (sorted by 128-node
    dst tile) -> indicator one-hot matmul scatter-adds each 128-edge chunk
    into the dst tile's PSUM accumulator -> epilogue.
  - int16 gather indices can only address 32767 rows, so nodes map into the
    two 25000-row tables: node v -> table (v%6250)//3125,
    row (v//6250)*3125 + (v%6250)%3125. Each tile's edges are grouped by
    table, each group padded to a multiple of 128 edges with index 0 /
    dst 255 (the indicator kills padding contributions).
  - dma_gather is capped at 1024 indices/instruction (SWDGE ring), so
    gathers are packed 8 chunks each and spread over 4 SWDGE queues.
"""

import numpy as np

N_NODES = 50000
N_EDGES = 800000
IN_CH = 128
HID = 64
OUT = 64
N_CORES = 8
PER_CORE = N_NODES // N_CORES          # 6250
N_TILES = (PER_CORE + 127) // 128      # 49
HALF_LOC = PER_CORE // 2               # 3125
TAB_ROWS = N_CORES * HALF_LOC          # 25000
PAD_DST = 255.0

_compiled_cache = {}


def _preprocess(edge_index: np.ndarray):
    """Host-side graph preprocessing -> per-core index/dst arrays + caps."""
    src = edge_index[0].astype(np.int64)
    dst = edge_index[1].astype(np.int64)

    deg = np.bincount(dst, minlength=N_NODES).astype(np.float64) + 1.0
    dis = (1.0 / np.sqrt(deg)).astype(np.float32)

    # table mapping: node v -> (half, row)
    src_core = src // PER_CORE
    src_r = src % PER_CORE
    half = (src_r >= HALF_LOC).astype(np.int64)
    tab_row = src_core * HALF_LOC + (src_r % HALF_LOC)

    core = dst // PER_CORE
    tile = (dst - core * PER_CORE) // 128
    order = np.lexsort((src, half, tile, core))
    row_s, dst_s = tab_row[order], dst[order]
    core_s, tile_s, half_s = core[order], tile[order], half[order]

    gid = (core_s * N_TILES + tile_s) * 2 + half_s
    counts = np.bincount(gid, minlength=N_CORES * N_TILES * 2).reshape(
        N_CORES, N_TILES, 2
    )
    cap128 = lambda x: max(128, int(-(-x // 128) * 128))
    cap_lo = cap128(counts[:, :, 0].max())
    cap_hi = cap128(counts[:, :, 1].max())
    c_lo, c_hi = cap_lo // 128, cap_hi // 128
    c_t = c_lo + c_hi

    starts = np.zeros(N_CORES * N_TILES * 2 + 1, dtype=np.int64)
    np.cumsum(counts.reshape(-1), out=starts[1:])

    per_core = []
    for c in range(N_CORES):
        idx_lo = np.zeros((N_TILES, cap_lo), dtype=np.int16)
        idx_hi = np.zeros((N_TILES, cap_hi), dtype=np.int16)
        dstc = np.full((N_TILES, c_t, 128), PAD_DST, dtype=np.float32)
        for t in range(N_TILES):
            g = (c * N_TILES + t) * 2
            n_lo = counts[c, t, 0]
            n_hi = counts[c, t, 1]
            s0 = starts[g]
            s1 = starts[g + 1]
            idx_lo[t, :n_lo] = row_s[s0:s0 + n_lo]
            idx_hi[t, :n_hi] = row_s[s1:s1 + n_hi]
            dloc = np.concatenate(
                [
                    dst_s[s0:s0 + n_lo] - c * PER_CORE - t * 128,
                    np.full(cap_lo - n_lo, PAD_DST),
                    dst_s[s1:s1 + n_hi] - c * PER_CORE - t * 128,
                    np.full(cap_hi - n_hi, PAD_DST),
                ]
            ).astype(np.float32)
            dstc[t] = dloc.reshape(c_t, 128)

        def wrap(a):  # [T, cap] -> [128, T*cap//16]
            w = a.reshape(N_TILES, -1, 16).transpose(2, 0, 1).reshape(16, -1)
            return np.tile(w, (8, 1)).copy()

        per_core.append(
            dict(
                idx_lo=wrap(idx_lo),
                idx_hi=wrap(idx_hi),
                dstc=dstc.transpose(2, 0, 1).reshape(128, -1).copy(),
            )
        )
    return dis, per_core, cap_lo, cap_hi


def _build(cap_lo, cap_hi, do_gather=True, do_ind=True, do_mm=True):
    import concourse.bacc as bacc
    import concourse.mybir as mybir
    import concourse.tile as tile
    from concourse.bass import ds, ts

    c_lo, c_hi = cap_lo // 128, cap_hi // 128
    c_t = c_lo + c_hi
    f32 = mybir.dt.float32

    nc = bacc.Bacc("TRN2", target_bir_lowering=False, debug=False,
                   num_devices=N_CORES, dynamic_dma_scratch_size=65536,
                   num_swdge_queues=4)

    # I/O
    xT_d = nc.dram_tensor("xT", [IN_CH, PER_CORE], f32, kind="ExternalInput")
    w1_d = nc.dram_tensor("w1", [IN_CH, HID], f32, kind="ExternalInput")
    w2_d = nc.dram_tensor("w2", [HID, OUT], f32, kind="ExternalInput")
    b1_d = nc.dram_tensor("b1", [1, HID], f32, kind="ExternalInput")
    b2_d = nc.dram_tensor("b2", [1, OUT], f32, kind="ExternalInput")
    dis_d = nc.dram_tensor("dis_t", [128, N_TILES], f32, kind="ExternalInput")
    ixlo_d = nc.dram_tensor("idx_lo", [128, N_TILES * cap_lo // 16],
                            mybir.dt.int16, kind="ExternalInput")
    ixhi_d = nc.dram_tensor("idx_hi", [128, N_TILES * cap_hi // 16],
                            mybir.dt.int16, kind="ExternalInput")
    dstc_d = nc.dram_tensor("dstc", [128, N_TILES * c_t], f32,
                            kind="ExternalInput")
    out_d = nc.dram_tensor("out_local", [PER_CORE, OUT], f32,
                           kind="ExternalOutput")

    # internal DRAM: per-layer half bounces + half tables
    bnc = {}
    tab = {}
    for layer, ch in ((1, HID), (2, OUT)):
        for s in ("lo", "hi"):
            bnc[layer, s] = nc.dram_tensor(f"bounce{layer}{s}",
                                           [HALF_LOC, ch], f32,
                                           kind="Internal")
            tab[layer, s] = nc.dram_tensor(f"table{layer}{s}",
                                           [TAB_ROWS, ch], f32,
                                           kind="Internal",
                                           addr_space="Shared")

    iota_np = np.tile(np.arange(128, dtype=np.float32), (128, 1))
    ident_np = np.eye(128, dtype=np.float32)
    iota_d = nc.inline_tensor(iota_np, name="iota128")
    ident_d = nc.inline_tensor(ident_np, name="ident128")

    with tile.TileContext(nc) as tc:
        with (
            tc.tile_pool(name="const", bufs=1) as cpool,
            tc.tile_pool(name="state", bufs=1) as spool,
            tc.tile_pool(name="work", bufs=3) as wpool,
            tc.tile_pool(name="gath", bufs=10) as gpool,
            tc.tile_pool(name="ind", bufs=3) as ipool,
            tc.tile_pool(name="psA", bufs=2, space="PSUM") as psA,
            tc.tile_pool(name="psB", bufs=4, space="PSUM") as psB,
            tc.tile_pool(name="psT", bufs=2, space="PSUM") as psT,
        ):
            # ---- constants / inputs to SBUF ----
            iota_sb = cpool.tile([128, 128], f32, tag="iota")
            nc.sync.dma_start(iota_sb[:], iota_d[:])
            ident_sb = cpool.tile([128, 128], f32, tag="ident")
            nc.sync.dma_start(ident_sb[:], ident_d[:])
            w1_sb = cpool.tile([IN_CH, HID], f32, tag="w1")
            nc.sync.dma_start(w1_sb[:], w1_d[:])
            w2_sb = cpool.tile([HID, OUT], f32, tag="w2")
            nc.sync.dma_start(w2_sb[:], w2_d[:])
            dis_sb = cpool.tile([128, N_TILES], f32, tag="dis")
            nc.sync.dma_start(dis_sb[:], dis_d[:])
            b1_row = cpool.tile([1, HID], f32, tag="b1r")
            nc.sync.dma_start(b1_row[:], b1_d[:])
            b2_row = cpool.tile([1, OUT], f32, tag="b2r")
            nc.sync.dma_start(b2_row[:], b2_d[:])
            b1_bc = cpool.tile([128, HID], f32, tag="b1b")
            nc.gpsimd.partition_broadcast(b1_bc[:], b1_row[:])
            b2_bc = cpool.tile([128, OUT], f32, tag="b2b")
            nc.gpsimd.partition_broadcast(b2_bc[:], b2_row[:])
            xT_sb = cpool.tile([IN_CH, PER_CORE], f32, tag="xT")
            nc.sync.dma_start(xT_sb[:], xT_d[:])
            ixlo_sb = cpool.tile([128, N_TILES * cap_lo // 16], mybir.dt.int16,
                                 tag="ixlo")
            nc.sync.dma_start(ixlo_sb[:], ixlo_d[:])
            ixhi_sb = cpool.tile([128, N_TILES * cap_hi // 16], mybir.dt.int16,
                                 tag="ixhi")
            nc.sync.dma_start(ixhi_sb[:], ixhi_d[:])
            dstc_sb = cpool.tile([128, N_TILES * c_t], f32, tag="dstc")
            nc.sync.dma_start(dstc_sb[:], dstc_d[:])

            # per-tile state tiles (fine-grained cross-phase deps)
            s1_t = [spool.tile([128, HID], f32, tag=f"s1_{t}", name=f"s1_{t}")
                    for t in range(N_TILES)]
            s2_t = [spool.tile([128, OUT], f32, tag=f"s2_{t}", name=f"s2_{t}")
                    for t in range(N_TILES)]
            h1_t = [spool.tile([128, HID], f32, tag=f"h1_{t}", name=f"h1_{t}")
                    for t in range(N_TILES)]
            nc.vector.memset(h1_t[N_TILES - 1][:], 0.0)

            def bounce_store(layer, t, nt, src_tile):
                """store [nt,ch] tile t rows into the lo/hi half bounces."""
                r0 = t * 128
                r1 = r0 + nt
                if r1 <= HALF_LOC:
                    nc.sync.dma_start(bnc[layer, "lo"][ds(r0, nt), :],
                                      src_tile[:nt, :])
                elif r0 >= HALF_LOC:
                    nc.sync.dma_start(bnc[layer, "hi"][ds(r0 - HALF_LOC, nt), :],
                                      src_tile[:nt, :])
                else:
                    n_a = HALF_LOC - r0
                    nc.sync.dma_start(bnc[layer, "lo"][ds(r0, n_a), :],
                                      src_tile[:n_a, :])
                    nc.sync.dma_start(bnc[layer, "hi"][ds(0, nt - n_a), :],
                                      src_tile[n_a:nt, :])

            def all_gather(layer, s):
                nc.gpsimd.collective_compute(
                    "AllGather", mybir.AluOpType.bypass,
                    replica_groups=[list(range(N_CORES))],
                    ins=[bnc[layer, s][:]], outs=[tab[layer, s][:]])

            def phase_a1(t, nt):
                """x@W1 -> hp=dis*xW1 -> s1=dis*hp+b1, store to bounce."""
                ps = psA.tile([128, HID], f32, tag="psa")
                nc.tensor.matmul(ps[:nt, :], xT_sb[:, ds(t * 128, nt)], w1_sb[:],
                                 start=True, stop=True)
                hp = wpool.tile([128, HID], f32, tag="hp")
                dcol = dis_sb[:nt, t:t + 1]
                nc.vector.tensor_scalar(hp[:nt, :], ps[:nt, :], dcol, None,
                                        mybir.AluOpType.mult)
                nc.vector.scalar_tensor_tensor(
                    s1_t[t][:nt, :], hp[:nt, :], dcol, b1_bc[:nt, :],
                    mybir.AluOpType.mult, mybir.AluOpType.add)
                bounce_store(1, t, nt, hp)

            def phase_a2(t, nt):
                """h1 tile -> transpose -> @W2 -> gp=dis*G -> s2, bounce."""
                pt = psT.tile([HID, 128], f32, tag="pst")
                nc.tensor.transpose(pt[:], h1_t[t][:], ident_sb[:])
                hT = wpool.tile([HID, 128], f32, tag="hT")
                nc.vector.tensor_copy(hT[:], pt[:])
                ps = psA.tile([128, OUT], f32, tag="psa")
                nc.tensor.matmul(ps[:], hT[:], w2_sb[:], start=True, stop=True)
                gp = wpool.tile([128, OUT], f32, tag="gp")
                dcol = dis_sb[:nt, t:t + 1]
                nc.vector.tensor_scalar(gp[:nt, :], ps[:nt, :], dcol, None,
                                        mybir.AluOpType.mult)
                nc.vector.scalar_tensor_tensor(
                    s2_t[t][:nt, :], gp[:nt, :], dcol, b2_bc[:nt, :],
                    mybir.AluOpType.mult, mybir.AluOpType.add)
                bounce_store(2, t, nt, gp)

            IDXG = 1024            # hard ucode cap per dma_gather
            CPG = IDXG // 128      # chunks per gather

            def phase_b(layer, ch, tile_done=None):
                """gather + indicator matmul scatter + epilogue."""
                gtiles = {"lo": {}, "hi": {}}
                qctr = [0]
                streams = {
                    "lo": (tab[layer, "lo"], ixlo_sb, N_TILES * c_lo),
                    "hi": (tab[layer, "hi"], ixhi_sb, N_TILES * c_hi),
                }

                def get_gather(stream, g):
                    if g in gtiles[stream]:
                        return gtiles[stream][g]
                    table_d, ix_sb, total = streams[stream]
                    n_ch = min(CPG, total - g * CPG)
                    tl = gpool.tile([128, CPG, ch], f32, tag="g" + stream)
                    if do_gather:
                        nc.gpsimd.dma_gather(
                            out_ap=tl[:, 0:n_ch, :],
                            in_ap=table_d[:],
                            idxs_ap=ix_sb[:, ds(g * IDXG // 16, n_ch * 8)],
                            num_idxs=n_ch * 128,
                            num_idxs_reg=n_ch * 128,
                            elem_size=ch,
                            queue_num=qctr[0] % 4,
                        )
                        qctr[0] += 1
                    else:
                        nc.vector.memset(tl[:], 0.0)
                    gtiles[stream][g] = tl
                    return tl

                for t in range(N_TILES):
                    nt = min(128, PER_CORE - t * 128)
                    ind = ipool.tile([128, c_t * 128], f32, tag="ind")
                    if do_ind:
                        for k in range(c_t):
                            nc.any.tensor_scalar(
                                ind[:, ts(k, 128)], iota_sb[:],
                                dstc_sb[:, t * c_t + k:t * c_t + k + 1],
                                None, mybir.AluOpType.is_equal)
                    else:
                        nc.vector.memset(ind[:], 0.0)
                    ps = psB.tile([128, ch], f32, tag="psb")
                    if do_mm:
                        for j in range(c_lo):
                            g, slot = divmod(t * c_lo + j, CPG)
                            tl = get_gather("lo", g)
                            nc.tensor.matmul(ps[:], ind[:, ts(j, 128)],
                                             tl[:, slot, :],
                                             start=(j == 0), stop=False)
                        for j in range(c_hi):
                            g, slot = divmod(t * c_hi + j, CPG)
                            tl = get_gather("hi", g)
                            nc.tensor.matmul(ps[:], ind[:, ts(c_lo + j, 128)],
                                             tl[:, slot, :],
                                             start=False, stop=(j == c_hi - 1))
                    else:
                        tl = get_gather("lo", (t * c_lo) // CPG)
                        nc.tensor.matmul(ps[:], ind[:, ts(0, 128)],
                                         tl[:, (t * c_lo) % CPG, :],
                                         start=True, stop=True)
                    dcol = dis_sb[:nt, t:t + 1]
                    if layer == 1:
                        nc.vector.scalar_tensor_tensor(
                            h1_t[t][:nt, :], ps[:nt, :], dcol, s1_t[t][:nt, :],
                            mybir.AluOpType.mult, mybir.AluOpType.add)
                        nc.scalar.activation(
                            h1_t[t][:nt, :], h1_t[t][:nt, :],
                            mybir.ActivationFunctionType.Relu)
                    else:
                        ot = wpool.tile([128, OUT], f32, tag="ot")
                        nc.vector.scalar_tensor_tensor(
                            ot[:nt, :], ps[:nt, :], dcol, s2_t[t][:nt, :],
                            mybir.AluOpType.mult, mybir.AluOpType.add)
                        nc.sync.dma_start(out_d[ds(t * 128, nt), :],
                                          ot[:nt, :])
                    if tile_done is not None:
                        tile_done(t, nt)

            # ---------- layer 1 ----------
            LAST_LO_TILE = (HALF_LOC - 1) // 128  # 24
            for t in range(N_TILES):
                phase_a1(t, min(128, PER_CORE - t * 128))
                if t == LAST_LO_TILE:
                    all_gather(1, "lo")
            all_gather(1, "hi")

            # layer-2 phase A runs per-tile as layer-1 phase B finishes tiles
            def l1_done(t, nt):
                phase_a2(t, nt)
                if t == LAST_LO_TILE:
                    all_gather(2, "lo")
                elif t == N_TILES - 1:
                    all_gather(2, "hi")

            phase_b(1, HID, tile_done=l1_done)

            # ---------- layer 2 ----------
            phase_b(2, OUT)

    nc.compile()
    return nc


def _make_in_maps(x, W1, b1, W2, b2, dis, per_core):
    in_maps = []
    for c in range(N_CORES):
        dis_c = np.zeros(N_TILES * 128, dtype=np.float32)
        dis_c[:PER_CORE] = dis[c * PER_CORE:(c + 1) * PER_CORE]
        in_maps.append(
            {
                "xT": np.ascontiguousarray(
                    x[c * PER_CORE:(c + 1) * PER_CORE].T),
                "w1": np.ascontiguousarray(W1),
                "w2": np.ascontiguousarray(W2),
                "b1": np.ascontiguousarray(b1.reshape(1, -1)),
                "b2": np.ascontiguousarray(b2.reshape(1, -1)),
                "dis_t": np.ascontiguousarray(
                    dis_c.reshape(N_TILES, 128).T),
                "idx_lo": per_core[c]["idx_lo"],
                "idx_hi": per_core[c]["idx_hi"],
                "dstc": per_core[c]["dstc"],
            }
        )
    return in_maps


def run(x, edge_index, W1, b1, W2, b2, trace=False):
    from concourse.bass_utils import run_bass_kernel_spmd

    x = np.asarray(x, dtype=np.float32)
    edge_index = np.asarray(edge_index)
    W1 = np.asarray(W1, dtype=np.float32)
    b1 = np.asarray(b1, dtype=np.float32)
    W2 = np.asarray(W2, dtype=np.float32)
    b2 = np.asarray(b2, dtype=np.float32)

    dis, per_core, cap_lo, cap_hi = _preprocess(edge_index)
    key = (cap_lo, cap_hi)
    if key not in _compiled_cache:
        _compiled_cache[key] = _build(cap_lo, cap_hi)
    nc = _compiled_cache[key]
    in_maps = _make_in_maps(x, W1, b1, W2, b2, dis, per_core)
    res = run_bass_kernel_spmd(nc, in_maps, core_ids=list(range(N_CORES)),
                               trace=trace)
    out = np.concatenate([res.results[c]["out_local"] for c in range(N_CORES)],
                         axis=0)
    return out, res


def kernel(x, edge_index, W1, b1, W2, b2):
    out, _ = run(x, edge_index, W1, b1, W2, b2, trace=False)
    return out


# revision 25
# speedup vs baseline: 1.1324x; 1.1324x over previous
"""GCN 2-layer (gcn_norm) SPMD Bass kernel for 8 TRN2 NeuronCores.

Strategy (node partition + edge partition by destination):
  - nodes sharded 6250/core; edges assigned to the core owning their dst.
  - layer math: out = dis * (sum_{e->v} dis[src]*h[src]) + dis^2*h_v + b
    with dis = deg^-1/2 (deg includes self-loop), h = x@W.
  - per layer: local projection -> scale by dis -> two half AllGathers
    (first/second half of each core's rows) into [25000,ch] tables in each
    core's HBM -> dma_gather rows for the core's edges (sorted by 128-node
    dst tile) -> indicator one-hot matmul scatter-adds each 128-edge chunk
    into the dst tile's PSUM accumulator -> epilogue.
  - int16 gather indices can only address 32767 rows, so nodes map into the
    two 25000-row tables: node v -> table (v%6250)//3125,
    row (v//6250)*3125 + (v%6250)%3125. Each tile's edges are grouped by
    table, each group padded to a multiple of 128 edges with index 0 /
    dst 255 (the indicator kills padding contributions).
  - dma_gather is capped at 1024 indices/instruction (SWDGE ring), so
    gathers are packed 8 chunks each and spread over 4 SWDGE queues.
"""

import numpy as np

N_NODES = 50000
N_EDGES = 800000
IN_CH = 128
HID = 64
OUT = 64
N_CORES = 8
PER_CORE = N_NODES // N_CORES          # 6250
N_TILES = (PER_CORE + 127) // 128      # 49
HALF_LOC = PER_CORE // 2               # 3125
TAB_ROWS = N_CORES * HALF_LOC          # 25000
PAD_DST = 255.0

_compiled_cache = {}


def _preprocess(edge_index: np.ndarray):
    """Host-side graph preprocessing -> per-core index/dst arrays + caps."""
    src = edge_index[0].astype(np.int64)
    dst = edge_index[1].astype(np.int64)

    deg = np.bincount(dst, minlength=N_NODES).astype(np.float64) + 1.0
    dis = (1.0 / np.sqrt(deg)).astype(np.float32)

    # table mapping: node v -> (half, row)
    src_core = src // PER_CORE
    src_r = src % PER_CORE
    half = (src_r >= HALF_LOC).astype(np.int64)
    tab_row = src_core * HALF_LOC + (src_r % HALF_LOC)

    core = dst // PER_CORE
    tile = (dst - core * PER_CORE) // 128
    order = np.lexsort((src, half, tile, core))
    row_s, dst_s = tab_row[order], dst[order]
    core_s, tile_s, half_s = core[order], tile[order], half[order]

    gid = (core_s * N_TILES + tile_s) * 2 + half_s
    counts = np.bincount(gid, minlength=N_CORES * N_TILES * 2).reshape(
        N_CORES, N_TILES, 2
    )
    cap128 = lambda x: max(128, int(-(-x // 128) * 128))
    cap_lo = cap128(counts[:, :, 0].max())
    cap_hi = cap128(counts[:, :, 1].max())
    c_lo, c_hi = cap_lo // 128, cap_hi // 128
    c_t = c_lo + c_hi

    starts = np.zeros(N_CORES * N_TILES * 2 + 1, dtype=np.int64)
    np.cumsum(counts.reshape(-1), out=starts[1:])

    per_core = []
    for c in range(N_CORES):
        idx_lo = np.zeros((N_TILES, cap_lo), dtype=np.int16)
        idx_hi = np.zeros((N_TILES, cap_hi), dtype=np.int16)
        dstc = np.full((N_TILES, c_t, 128), PAD_DST, dtype=np.float32)
        for t in range(N_TILES):
            g = (c * N_TILES + t) * 2
            n_lo = counts[c, t, 0]
            n_hi = counts[c, t, 1]
            s0 = starts[g]
            s1 = starts[g + 1]
            idx_lo[t, :n_lo] = row_s[s0:s0 + n_lo]
            idx_hi[t, :n_hi] = row_s[s1:s1 + n_hi]
            dloc = np.concatenate(
                [
                    dst_s[s0:s0 + n_lo] - c * PER_CORE - t * 128,
                    np.full(cap_lo - n_lo, PAD_DST),
                    dst_s[s1:s1 + n_hi] - c * PER_CORE - t * 128,
                    np.full(cap_hi - n_hi, PAD_DST),
                ]
            ).astype(np.float32)
            dstc[t] = dloc.reshape(c_t, 128)

        def wrap(a):  # [T, cap] -> [128, T*cap//16]
            w = a.reshape(N_TILES, -1, 16).transpose(2, 0, 1).reshape(16, -1)
            return np.tile(w, (8, 1)).copy()

        per_core.append(
            dict(
                idx_lo=wrap(idx_lo),
                idx_hi=wrap(idx_hi),
                dstc=dstc.transpose(2, 0, 1).reshape(128, -1).copy(),
            )
        )
    return dis, per_core, cap_lo, cap_hi


def _build(cap_lo, cap_hi, do_gather=True, do_ind=True, do_mm=True):
    import concourse.bacc as bacc
    import concourse.mybir as mybir
    import concourse.tile as tile
    from concourse.bass import ds, ts

    c_lo, c_hi = cap_lo // 128, cap_hi // 128
    c_t = c_lo + c_hi
    f32 = mybir.dt.float32

    nc = bacc.Bacc("TRN2", target_bir_lowering=False, debug=False,
                   num_devices=N_CORES, dynamic_dma_scratch_size=65536,
                   num_swdge_queues=4)

    # I/O
    xT_d = nc.dram_tensor("xT", [IN_CH, PER_CORE], f32, kind="ExternalInput")
    w1_d = nc.dram_tensor("w1", [IN_CH, HID], f32, kind="ExternalInput")
    w2_d = nc.dram_tensor("w2", [HID, OUT], f32, kind="ExternalInput")
    b1_d = nc.dram_tensor("b1", [1, HID], f32, kind="ExternalInput")
    b2_d = nc.dram_tensor("b2", [1, OUT], f32, kind="ExternalInput")
    dis_d = nc.dram_tensor("dis_t", [128, N_TILES], f32, kind="ExternalInput")
    ixlo_d = nc.dram_tensor("idx_lo", [128, N_TILES * cap_lo // 16],
                            mybir.dt.int16, kind="ExternalInput")
    ixhi_d = nc.dram_tensor("idx_hi", [128, N_TILES * cap_hi // 16],
                            mybir.dt.int16, kind="ExternalInput")
    dstc_d = nc.dram_tensor("dstc", [128, N_TILES * c_t], f32,
                            kind="ExternalInput")
    out_d = nc.dram_tensor("out_local", [PER_CORE, OUT], f32,
                           kind="ExternalOutput")

    # internal DRAM: per-layer half bounces + half tables
    bnc = {}
    tab = {}
    for layer, ch in ((1, HID), (2, OUT)):
        for s in ("lo", "hi"):
            bnc[layer, s] = nc.dram_tensor(f"bounce{layer}{s}",
                                           [HALF_LOC, ch], f32,
                                           kind="Internal")
            tab[layer, s] = nc.dram_tensor(f"table{layer}{s}",
                                           [TAB_ROWS, ch], f32,
                                           kind="Internal",
                                           addr_space="Shared")

    iota_np = np.tile(np.arange(128, dtype=np.float32), (128, 1))
    ident_np = np.eye(128, dtype=np.float32)
    iota_d = nc.inline_tensor(iota_np, name="iota128")
    ident_d = nc.inline_tensor(ident_np, name="ident128")

    with tile.TileContext(nc) as tc:
        with (
            tc.tile_pool(name="const", bufs=1) as cpool,
            tc.tile_pool(name="state", bufs=1) as spool,
            tc.tile_pool(name="work", bufs=3) as wpool,
            tc.tile_pool(name="gath", bufs=9) as gpool,
            tc.tile_pool(name="ind", bufs=4) as ipool,
            tc.tile_pool(name="psA", bufs=2, space="PSUM") as psA,
            tc.tile_pool(name="psB", bufs=4, space="PSUM") as psB,
            tc.tile_pool(name="psT", bufs=2, space="PSUM") as psT,
        ):
            # ---- constants / inputs to SBUF ----
            iota_sb = cpool.tile([128, 128], f32, tag="iota")
            nc.sync.dma_start(iota_sb[:], iota_d[:])
            ident_sb = cpool.tile([128, 128], f32, tag="ident")
            nc.sync.dma_start(ident_sb[:], ident_d[:])
            w1_sb = cpool.tile([IN_CH, HID], f32, tag="w1")
            nc.sync.dma_start(w1_sb[:], w1_d[:])
            w2_sb = cpool.tile([HID, OUT], f32, tag="w2")
            nc.sync.dma_start(w2_sb[:], w2_d[:])
            dis_sb = cpool.tile([128, N_TILES], f32, tag="dis")
            nc.sync.dma_start(dis_sb[:], dis_d[:])
            b1_row = cpool.tile([1, HID], f32, tag="b1r")
            nc.sync.dma_start(b1_row[:], b1_d[:])
            b2_row = cpool.tile([1, OUT], f32, tag="b2r")
            nc.sync.dma_start(b2_row[:], b2_d[:])
            b1_bc = cpool.tile([128, HID], f32, tag="b1b")
            nc.gpsimd.partition_broadcast(b1_bc[:], b1_row[:])
            b2_bc = cpool.tile([128, OUT], f32, tag="b2b")
            nc.gpsimd.partition_broadcast(b2_bc[:], b2_row[:])
            xT_sb = cpool.tile([IN_CH, PER_CORE], f32, tag="xT")
            nc.sync.dma_start(xT_sb[:], xT_d[:])
            ixlo_sb = cpool.tile([128, N_TILES * cap_lo // 16], mybir.dt.int16,
                                 tag="ixlo")
            nc.sync.dma_start(ixlo_sb[:], ixlo_d[:])
            ixhi_sb = cpool.tile([128, N_TILES * cap_hi // 16], mybir.dt.int16,
                                 tag="ixhi")
            nc.sync.dma_start(ixhi_sb[:], ixhi_d[:])
            dstc_sb = cpool.tile([128, N_TILES * c_t], f32, tag="dstc")
            nc.sync.dma_start(dstc_sb[:], dstc_d[:])


            # per-tile state tiles (fine-grained cross-phase deps)
            s1_t = [spool.tile([128, HID], f32, tag=f"s1_{t}", name=f"s1_{t}")
                    for t in range(N_TILES)]
            s2_t = [spool.tile([128, OUT], f32, tag=f"s2_{t}", name=f"s2_{t}")
                    for t in range(N_TILES)]
            h1_t = [spool.tile([128, HID], f32, tag=f"h1_{t}", name=f"h1_{t}")
                    for t in range(N_TILES)]
            nc.vector.memset(h1_t[N_TILES - 1][:], 0.0)

            def bounce_store(layer, t, nt, src_tile):
                """store [nt,ch] tile t rows into the lo/hi half bounces."""
                r0 = t * 128
                r1 = r0 + nt
                if r1 <= HALF_LOC:
                    nc.sync.dma_start(bnc[layer, "lo"][ds(r0, nt), :],
                                      src_tile[:nt, :])
                elif r0 >= HALF_LOC:
                    nc.sync.dma_start(bnc[layer, "hi"][ds(r0 - HALF_LOC, nt), :],
                                      src_tile[:nt, :])
                else:
                    n_a = HALF_LOC - r0
                    nc.sync.dma_start(bnc[layer, "lo"][ds(r0, n_a), :],
                                      src_tile[:n_a, :])
                    nc.sync.dma_start(bnc[layer, "hi"][ds(0, nt - n_a), :],
                                      src_tile[n_a:nt, :])

            def all_gather(layer, s):
                nc.gpsimd.collective_compute(
                    "AllGather", mybir.AluOpType.bypass,
                    replica_groups=[list(range(N_CORES))],
                    ins=[bnc[layer, s][:]], outs=[tab[layer, s][:]])

            def phase_a1(t, nt):
                """x@W1 -> hp=dis*xW1 -> s1=dis*hp+b1, store to bounce."""
                ps = psA.tile([128, HID], f32, tag="psa")
                nc.tensor.matmul(ps[:nt, :], xT_sb[:, ds(t * 128, nt)], w1_sb[:],
                                 start=True, stop=True)
                hp = wpool.tile([128, HID], f32, tag="hp")
                dcol = dis_sb[:nt, t:t + 1]
                nc.vector.tensor_scalar(hp[:nt, :], ps[:nt, :], dcol, None,
                                        mybir.AluOpType.mult)
                nc.vector.scalar_tensor_tensor(
                    s1_t[t][:nt, :], hp[:nt, :], dcol, b1_bc[:nt, :],
                    mybir.AluOpType.mult, mybir.AluOpType.add)
                bounce_store(1, t, nt, hp)

            def phase_a2(t, nt):
                """h1 tile -> transpose -> @W2 -> gp=dis*G -> s2, bounce."""
                pt = psT.tile([HID, 128], f32, tag="pst")
                nc.tensor.transpose(pt[:], h1_t[t][:], ident_sb[:])
                hT = wpool.tile([HID, 128], f32, tag="hT")
                nc.vector.tensor_copy(hT[:], pt[:])
                ps = psA.tile([128, OUT], f32, tag="psa")
                nc.tensor.matmul(ps[:], hT[:], w2_sb[:], start=True, stop=True)
                gp = wpool.tile([128, OUT], f32, tag="gp")
                dcol = dis_sb[:nt, t:t + 1]
                nc.vector.tensor_scalar(gp[:nt, :], ps[:nt, :], dcol, None,
                                        mybir.AluOpType.mult)
                nc.vector.scalar_tensor_tensor(
                    s2_t[t][:nt, :], gp[:nt, :], dcol, b2_bc[:nt, :],
                    mybir.AluOpType.mult, mybir.AluOpType.add)
                bounce_store(2, t, nt, gp)

            IDXG = 1024            # hard ucode cap per dma_gather
            CPG = IDXG // 128      # chunks per gather

            def phase_b(layer, ch, tile_done=None):
                """gather + indicator matmul scatter + epilogue."""
                gtiles = {"lo": {}, "hi": {}}
                qctr = [0]
                streams = {
                    "lo": (tab[layer, "lo"], ixlo_sb, N_TILES * c_lo),
                    "hi": (tab[layer, "hi"], ixhi_sb, N_TILES * c_hi),
                }

                def get_gather(stream, g):
                    if g in gtiles[stream]:
                        return gtiles[stream][g]
                    table_d, ix_sb, total = streams[stream]
                    n_ch = min(CPG, total - g * CPG)
                    tl = gpool.tile([128, CPG, ch], f32, tag="g" + stream)
                    if do_gather:
                        nc.gpsimd.dma_gather(
                            out_ap=tl[:, 0:n_ch, :],
                            in_ap=table_d[:],
                            idxs_ap=ix_sb[:, ds(g * IDXG // 16, n_ch * 8)],
                            num_idxs=n_ch * 128,
                            num_idxs_reg=n_ch * 128,
                            elem_size=ch,
                            queue_num=qctr[0] % 4,
                        )
                        qctr[0] += 1
                    else:
                        nc.vector.memset(tl[:], 0.0)
                    gtiles[stream][g] = tl
                    return tl

                for t in range(N_TILES):
                    nt = min(128, PER_CORE - t * 128)
                    ind = ipool.tile([128, c_t * 128], f32, tag="ind")
                    if do_ind:
                        for k in range(c_t):
                            col = t * c_t + k
                            if k % 3 == 2:
                                # ACT path: ind = relu(1 - |dst - iota|)
                                nc.scalar.activation(
                                    ind[:, ts(k, 128)], iota_sb[:],
                                    mybir.ActivationFunctionType.Abs,
                                    bias=dstc_sb[:, col:col + 1], scale=-1.0)
                                nc.scalar.activation(
                                    ind[:, ts(k, 128)], ind[:, ts(k, 128)],
                                    mybir.ActivationFunctionType.Relu,
                                    bias=1.0, scale=-1.0)
                            else:
                                nc.vector.tensor_scalar(
                                    ind[:, ts(k, 128)], iota_sb[:],
                                    dstc_sb[:, col:col + 1],
                                    None, mybir.AluOpType.is_equal)
                    else:
                        nc.vector.memset(ind[:], 0.0)
                    ps = psB.tile([128, ch], f32, tag="psb")
                    if do_mm:
                        for j in range(c_lo):
                            g, slot = divmod(t * c_lo + j, CPG)
                            tl = get_gather("lo", g)
                            nc.tensor.matmul(ps[:], ind[:, ts(j, 128)],
                                             tl[:, slot, :],
                                             start=(j == 0), stop=False)
                        for j in range(c_hi):
                            g, slot = divmod(t * c_hi + j, CPG)
                            tl = get_gather("hi", g)
                            nc.tensor.matmul(ps[:], ind[:, ts(c_lo + j, 128)],
                                             tl[:, slot, :],
                                             start=False, stop=(j == c_hi - 1))
                    else:
                        tl = get_gather("lo", (t * c_lo) // CPG)
                        nc.tensor.matmul(ps[:], ind[:, ts(0, 128)],
                                         tl[:, (t * c_lo) % CPG, :],
                                         start=True, stop=True)
                    dcol = dis_sb[:nt, t:t + 1]
                    if layer == 1:
                        nc.vector.scalar_tensor_tensor(
                            h1_t[t][:nt, :], ps[:nt, :], dcol, s1_t[t][:nt, :],
                            mybir.AluOpType.mult, mybir.AluOpType.add)
                        nc.scalar.activation(
                            h1_t[t][:nt, :], h1_t[t][:nt, :],
                            mybir.ActivationFunctionType.Relu)
                    else:
                        ot = wpool.tile([128, OUT], f32, tag="ot")
                        nc.vector.scalar_tensor_tensor(
                            ot[:nt, :], ps[:nt, :], dcol, s2_t[t][:nt, :],
                            mybir.AluOpType.mult, mybir.AluOpType.add)
                        nc.sync.dma_start(out_d[ds(t * 128, nt), :],
                                          ot[:nt, :])
                    if tile_done is not None:
                        tile_done(t, nt)

            # ---------- layer 1 ----------
            LAST_LO_TILE = (HALF_LOC - 1) // 128  # 24
            for t in range(N_TILES):
                phase_a1(t, min(128, PER_CORE - t * 128))
            all_gather(1, "lo")
            all_gather(1, "hi")

            # layer-2 phase A runs per-tile as layer-1 phase B finishes tiles
            def l1_done(t, nt):
                phase_a2(t, nt)
                if t == LAST_LO_TILE:
                    all_gather(2, "lo")
                elif t == N_TILES - 1:
                    all_gather(2, "hi")

            phase_b(1, HID, tile_done=l1_done)

            # ---------- layer 2 ----------
            phase_b(2, OUT)

    nc.compile()
    return nc


def _make_in_maps(x, W1, b1, W2, b2, dis, per_core):
    in_maps = []
    for c in range(N_CORES):
        dis_c = np.zeros(N_TILES * 128, dtype=np.float32)
        dis_c[:PER_CORE] = dis[c * PER_CORE:(c + 1) * PER_CORE]
        in_maps.append(
            {
                "xT": np.ascontiguousarray(
                    x[c * PER_CORE:(c + 1) * PER_CORE].T),
                "w1": np.ascontiguousarray(W1),
                "w2": np.ascontiguousarray(W2),
                "b1": np.ascontiguousarray(b1.reshape(1, -1)),
                "b2": np.ascontiguousarray(b2.reshape(1, -1)),
                "dis_t": np.ascontiguousarray(
                    dis_c.reshape(N_TILES, 128).T),
                "idx_lo": per_core[c]["idx_lo"],
                "idx_hi": per_core[c]["idx_hi"],
                "dstc": per_core[c]["dstc"],
            }
        )
    return in_maps


def run(x, edge_index, W1, b1, W2, b2, trace=False):
    from concourse.bass_utils import run_bass_kernel_spmd

    x = np.asarray(x, dtype=np.float32)
    edge_index = np.asarray(edge_index)
    W1 = np.asarray(W1, dtype=np.float32)
    b1 = np.asarray(b1, dtype=np.float32)
    W2 = np.asarray(W2, dtype=np.float32)
    b2 = np.asarray(b2, dtype=np.float32)

    dis, per_core, cap_lo, cap_hi = _preprocess(edge_index)
    key = (cap_lo, cap_hi)
    if key not in _compiled_cache:
        _compiled_cache[key] = _build(cap_lo, cap_hi)
    nc = _compiled_cache[key]
    in_maps = _make_in_maps(x, W1, b1, W2, b2, dis, per_core)
    res = run_bass_kernel_spmd(nc, in_maps, core_ids=list(range(N_CORES)),
                               trace=trace)
    out = np.concatenate([res.results[c]["out_local"] for c in range(N_CORES)],
                         axis=0)
    return out, res


def kernel(x, edge_index, W1, b1, W2, b2):
    out, _ = run(x, edge_index, W1, b1, W2, b2, trace=False)
    return out
